# revision 12
# baseline (speedup 1.0000x reference)
"""Trainium2 Bass kernel for nn_Castle_34351148433552 (sparse_attention).

Sharding: 8 cores = 2 batches x 4 head-groups. Core c handles batch c//4,
heads 4*(c%4) .. 4*(c%4)+3. W_qkv is sliced column-wise per head group (with
the q-scale, and the silu-via-tanh 0.5 factor on vu, folded in on the host),
W_out row-wise. Each core emits TWO partial transposed projections ([1024,
2048] fp16, one per head pair); the host sums the 8 partials per batch and
transposes back.

All matmul operands are fp16 (1 PE row/cycle; fp32 accumulation in PSUM).
x^T and W_qkv stay SBUF-resident for the whole kernel. The Su / Sc / exp /
AV chain only computes the causal (i >= k, i >= j) free-ranges. The head
pair {0,1} output projection is emitted inside head 2 so only the {2,3}
pass trails the attention compute.

Device algorithm per (core, head), all in [d|c, n]-transposed layout:
  qkvT = (Wq_head^T x^T) via PE;  LT[j,k] = sigmoid(ku_j . qu_s_k) (j>k);
  T1T[j,i] = vu_j . qc_s_i (i>=j, pre-halved);  SuT'[k,i] = sum_j LT*T1T;
  silu(Su) = Su'*(1+tanh(Su')) (tanh shares the exp ACT table-set);
  scoresT = ScT - silu, causal fill -1e30;  expT;  AV via [vc|1] lhsT gives
  unnormalized out^T plus the softmax denominator in one PSUM accumulation;
  normalize (reciprocal_approx_fast + PE broadcast); project through W_out.
"""

import os
import sys

import numpy as np

for _p in ("/opt/trn_rl_repo", os.path.expanduser("~/.axon_site/_ro/trn_rl_repo")):
    if os.path.isdir(_p) and _p not in sys.path:
        sys.path.insert(0, _p)
        break

H, D, NTOK, DIM = 16, 64, 2048, 1024
P = 128
NB = NTOK // P  # 16 row blocks
GW = 512        # i/k group width
NG = NTOK // GW  # 4 groups
HPC = 4         # heads per core
NCORES = 8
WHEAD = 6 * D   # 384 qkv columns per head


def _lt_offsets():
    off = {}
    o = 0
    for J in range(NB):
        for kg in range(J // 4 + 1):
            w = GW if kg < J // 4 else (J % 4 + 1) * P
            off[(J, kg)] = (o, w)
            o += w
    return off, o


_NC_CACHE = None


def build_nc():
    global _NC_CACHE
    if _NC_CACHE is not None:
        return _NC_CACHE

    import concourse.mybir as mybir
    import concourse.tile as tile
    from concourse import bacc
    from concourse.masks import make_identity

    dt = mybir.dt
    F32 = dt.float32
    F32R = dt.float32r
    F16 = dt.float16
    AF = mybir.ActivationFunctionType
    ALU = mybir.AluOpType

    nc = bacc.Bacc(None, target_bir_lowering=False, debug=False)
    xT_d = nc.dram_tensor("xT", [DIM, NTOK], F16, kind="ExternalInput")
    wq_d = nc.dram_tensor("wq", [DIM, HPC * WHEAD], F16, kind="ExternalInput")
    wo_d = nc.dram_tensor("wo", [HPC * D, DIM], F16, kind="ExternalInput")
    out_d = [nc.dram_tensor(f"out{i}", [DIM, NTOK], F16, kind="ExternalOutput")
             for i in range(2)]

    lt_off, LTW = _lt_offsets()

    with tile.TileContext(nc) as tc:
        with (
            tc.tile_pool(name="const", bufs=1) as constp,
            tc.tile_pool(name="res", bufs=1) as resp,
            tc.tile_pool(name="work", bufs=2) as workp,
            tc.tile_pool(name="outsb", bufs=3) as outsbp,
            tc.tile_pool(name="ps", bufs=6, space="PSUM") as psp,
            tc.tile_pool(name="pavp", bufs=2, space="PSUM") as pavp,
        ):
            # ---------- constants ----------
            ident = constp.tile([P, P], F16, tag="ident")
            make_identity(nc, ident)
            # dmask[p, c] = 1.0 iff c >= p  (T1T diag block: keep i >= j)
            dmask = constp.tile([P, P], F16, tag="dmask")
            nc.gpsimd.memset(dmask, 1.0)
            nc.gpsimd.affine_select(
                out=dmask, in_=dmask, compare_op=ALU.is_ge, fill=0.0,
                base=0, pattern=[[1, P]], channel_multiplier=-1,
            )
            # smask[p, c] = 1.0 iff p > c  (LT diag block: keep j > k strictly)
            smask = constp.tile([P, P], F16, tag="smask")
            nc.gpsimd.memset(smask, 1.0)
            nc.gpsimd.affine_select(
                out=smask, in_=smask, compare_op=ALU.is_gt, fill=0.0,
                base=0, pattern=[[-1, P]], channel_multiplier=1,
            )
            # [vc | 1] stationary blocks, one [128, 65] slot per k-block.
            vc_ones = constp.tile([P, NB * (D + 1)], F16, tag="vco")
            ones_stage = constp.tile([P, NB], F16, tag="onesstage")
            nc.gpsimd.memset(ones_stage, 1.0)
            nc.vector.tensor_copy(
                vc_ones.rearrange("p (k c) -> p k c", c=D + 1)[:, :, D],
                ones_stage)

            # ---------- resident tiles ----------
            xres = resp.tile([P, 8 * NTOK], F16, tag="xres")
            wqres = resp.tile([P, 8 * HPC * WHEAD], F16, tag="wqres")
            # per-head transposed qkv (matmul lhsT/rhs base partitions must
            # match): t0=[qu|vu], t1=[ku|qc], t2=[vc|kc]
            qkvT = [resp.tile([P, NTOK], F16, tag=f"qkvT{i}", name=f"qkvT{i}")
                    for i in range(3)]
            LT = resp.tile([P, LTW], F16, tag="LT")
            T1T = resp.tile([P, NB * GW], F16, tag="T1T")
            attn = [resp.tile([P, NTOK], F16, tag=f"attn{i}", name=f"attn{i}")
                    for i in range(2)]
            wores = [resp.tile([P, DIM], F16, tag=f"wores{i}", name=f"wores{i}")
                     for i in range(2)]

            for i in range(2):
                nc.sync.dma_start(wores[i], wo_d[i * P:(i + 1) * P, :])
            # head-0 weights + first x chunk first so phase A starts early
            wqres3 = wqres.rearrange("p (kc w) -> p kc w", kc=8)
            wqd3 = wq_d.rearrange("(kc p) w -> p kc w", p=P)
            xres3 = xres.rearrange("p (kc n) -> p kc n", kc=8)
            xTd3 = xT_d.rearrange("(kc p) n -> p kc n", p=P)
            nc.sync.dma_start(wqres3[:, :, 0:WHEAD], wqd3[:, :, 0:WHEAD])
            nc.sync.dma_start(
                xres3[:, :, 0:GW], xTd3[:, :, 0:GW])
            for ng in range(1, NG):
                nc.sync.dma_start(
                    xres3[:, :, ng * GW:(ng + 1) * GW],
                    xTd3[:, :, ng * GW:(ng + 1) * GW])
            for h2 in range(1, HPC):
                nc.sync.dma_start(
                    wqres3[:, :, h2 * WHEAD:(h2 + 1) * WHEAD],
                    wqd3[:, :, h2 * WHEAD:(h2 + 1) * WHEAD])

            def emit_out_proj(pair, ngs=range(NG)):
                """Project attn[pair] through wores[pair] into out_d[pair]."""
                for ng in ngs:
                    for dt_ in range(8):
                        pp = psp.tile([P, GW], F32, tag="ps")
                        nc.tensor.matmul(
                            pp, wores[pair][:, dt_ * P:(dt_ + 1) * P],
                            attn[pair][:, ng * GW:(ng + 1) * GW],
                            start=True, stop=True)
                        ot = outsbp.tile([P, GW], F16, tag="ot")
                        if dt_ % 2 == 0:
                            nc.scalar.copy(ot, pp)
                        else:
                            nc.vector.tensor_copy(ot, pp)
                        nc.sync.dma_start(
                            out_d[pair][dt_ * P:(dt_ + 1) * P,
                                        ng * GW:(ng + 1) * GW], ot)

            for hh in range(HPC):
                # ---------- A: qkv projection for this head ----------
                for ng in range(NG):
                    ps = []
                    for ct in range(3):
                        ps.append(psp.tile([P, GW], F32, tag="ps", name=f"psq{ct}"))
                    for kc in range(8):
                        for ct in range(3):
                            wslice = wqres[
                                :, kc * HPC * WHEAD + hh * WHEAD + ct * P:
                                kc * HPC * WHEAD + hh * WHEAD + (ct + 1) * P]
                            nc.tensor.matmul(
                                ps[ct], wslice,
                                xres[:, kc * NTOK + ng * GW:
                                     kc * NTOK + (ng + 1) * GW],
                                start=(kc == 0), stop=(kc == 7),
                            )
                    for ct in range(3):
                        nc.scalar.copy(
                            qkvT[ct][:, ng * GW:(ng + 1) * GW], ps[ct])

                # out-projection of the finished head pair {0,1}, placed
                # after head-2 phase A so the PE never waits for it
                if hh == 2:
                    emit_out_proj(0)

                # ---------- C: LT = masked sigmoid(ku . qu_s) ----------
                # J-ascending so Su of early i-groups can start (subtile deps)
                for J in range(NB):
                    for kg in range(J // 4 + 1):
                        off, w = lt_off[(J, kg)]
                        pl = psp.tile([P, GW], F32, tag="ps")
                        nc.tensor.matmul(
                            pl[:, :w],
                            qkvT[1][0:D, J * P:(J + 1) * P],
                            qkvT[0][0:D, kg * GW: kg * GW + w],
                            start=True, stop=True,
                        )
                        nc.scalar.activation(
                            LT[:, off:off + w], pl[:, :w], AF.Sigmoid)
                        if kg == J // 4:
                            # strict j > k mask on the final (diagonal) block
                            nc.gpsimd.tensor_tensor(
                                LT[:, off + w - P:off + w],
                                LT[:, off + w - P:off + w],
                                smask, op=ALU.mult)

                # ---------- B: vc natural layout + ones columns ----------
                for kb in range(NB):
                    pt = psp.tile([P, D], F16, tag="ps")
                    nc.tensor.transpose(
                        pt, qkvT[2][0:D, kb * P:(kb + 1) * P], ident[0:D, 0:D])
                    nc.vector.tensor_copy(
                        vc_ones[:, kb * (D + 1): kb * (D + 1) + D], pt)

                # ---------- D: attention, per 512-wide i-group ----------
                avsH = workp.tile([D, NTOK], F16, tag="avsH")
                for ig in range(NG):
                    nblk = 4 * ig + 4

                    # head-3 ig>0: project the previous ig's finished
                    # attn[1] columns while this ig computes
                    if hh == 3 and ig > 0:
                        emit_out_proj(1, ngs=[ig - 1])

                    # T1T strips (term1 transposed, pre-halved); strip J only
                    # needs i >= j, so diag-group strips start at their block
                    for J in range(nblk):
                        o = max(J - 4 * ig, 0) * P  # first needed i-col
                        pt2 = psp.tile([P, GW], F32, tag="ps")
                        nc.tensor.matmul(
                            pt2[:, o:],
                            qkvT[0][D:2 * D, J * P:(J + 1) * P],
                            qkvT[1][D:2 * D, ig * GW + o:(ig + 1) * GW],
                            start=True, stop=True,
                        )
                        dst = T1T[:, J * GW:(J + 1) * GW]
                        if J < 4 * ig:
                            nc.vector.tensor_copy(dst, pt2)
                        else:
                            # diagonal 128-block: keep i >= j
                            nc.vector.tensor_tensor(
                                dst[:, o:o + P], pt2[:, o:o + P], dmask,
                                op=ALU.mult)
                            if o + P < GW:
                                nc.scalar.copy(dst[:, o + P:], pt2[:, o + P:])

                    # merged scores pass per k-block: Su' accumulate, Sc,
                    # silu-via-tanh, subtract, causal fill, exp, AV accum.
                    # Diagonal k-blocks only need i-cols >= k.
                    pav_t = pavp.tile([D + 1, GW], F32, tag="av")
                    LAG = 2
                    av_q = []

                    def emit_av(K, ext, ko):
                        nc.tensor.matmul(
                            pav_t[:, ko:],
                            vc_ones[:, K * (D + 1):(K + 1) * (D + 1)],
                            ext[:, ko:],
                            start=(K == 0), stop=(K == nblk - 1),
                        )

                    for K in range(nblk):
                        ko = max(K - 4 * ig, 0) * P  # first causal i-col
                        psu = psp.tile([P, GW], F32, tag="ps")
                        for J in range(K, nblk):
                            jo = max(J - 4 * ig, 0) * P
                            o_, _w = lt_off[(J, K // 4)]
                            nc.tensor.matmul(
                                psu[:, jo:],
                                LT[:, o_ + (K % 4) * P: o_ + (K % 4 + 1) * P],
                                T1T[:, J * GW + jo:(J + 1) * GW],
                                start=(J == K), stop=(J == nblk - 1),
                            )
                        psc = psp.tile([P, GW], F32, tag="ps")
                        nc.tensor.matmul(
                            psc[:, ko:],
                            qkvT[2][D:2 * D, K * P:(K + 1) * P],
                            qkvT[1][D:2 * D, ig * GW + ko:(ig + 1) * GW],
                            start=True, stop=True,
                        )
                        tnh = workp.tile([P, GW], F32, tag="tanh")
                        nc.scalar.activation(tnh[:, ko:], psu[:, ko:], AF.Tanh)
                        # silu(Su): tnh = (tnh + 1) * Su'
                        nc.vector.scalar_tensor_tensor(
                            out=tnh[:, ko:], in0=tnh[:, ko:], scalar=1.0,
                            in1=psu[:, ko:], op0=ALU.add, op1=ALU.mult,
                        )
                        sct = workp.tile([P, GW], F32, tag="sct")
                        nc.vector.tensor_tensor(
                            sct[:, ko:], psc[:, ko:], tnh[:, ko:],
                            op=ALU.subtract)
                        if K >= 4 * ig:
                            # partial diag 128-block: keep i >= k
                            nc.gpsimd.affine_select(
                                out=sct[:, ko:ko + P], in_=sct[:, ko:ko + P],
                                compare_op=ALU.is_ge, fill=-1e30,
                                base=0, pattern=[[1, P]], channel_multiplier=-1,
                            )
                        ext = workp.tile([P, GW], F16, tag="ext", bufs=4)
                        nc.scalar.activation(ext[:, ko:], sct[:, ko:], AF.Exp)
                        av_q.append((K, ext, ko))
                        if len(av_q) > LAG:
                            emit_av(*av_q.pop(0))
                    for item in av_q:
                        emit_av(*item)

                    # normalize: stage denom to SBUF (the custom-DVE
                    # reciprocal must not read PSUM), fast reciprocal, fp16,
                    # GpSimd partition-broadcast, all-SBUF fp16 multiply.
                    # The PE queue is never involved.
                    nc.scalar.copy(avsH[:, ig * GW:(ig + 1) * GW], pav_t[0:D, :])
                    dstrow = workp.tile([1, GW], F32, tag="dst", bufs=3)
                    nc.vector.tensor_copy(dstrow, pav_t[D:D + 1, :])
                    recip_t = workp.tile([1, GW], F32, tag="recip", bufs=3)
                    nc.vector.reciprocal_approx_fast(out=recip_t, in_=dstrow)
                    recip16 = workp.tile([1, GW], F16, tag="recip16", bufs=3)
                    nc.vector.tensor_copy(recip16, recip_t)
                    rbc = workp.tile([D, GW], F16, tag="rbc", bufs=3)
                    nc.gpsimd.partition_broadcast(rbc, recip16)
                    at = attn[hh // 2][(hh % 2) * D:(hh % 2 + 1) * D,
                                       ig * GW:(ig + 1) * GW]
                    nc.vector.tensor_tensor(
                        at, avsH[:, ig * GW:(ig + 1) * GW], rbc, op=ALU.mult)

            # ---------- E: trailing columns of head pair {2,3} ----------
            emit_out_proj(1, ngs=[NG - 1])

    nc.compile()
    _NC_CACHE = nc
    return nc


def shard_inputs(x, W_qkv, W_out):
    """Host-side sharding: per-core input dicts (fp16)."""
    x = np.asarray(x, np.float32)
    W_qkv = np.asarray(W_qkv, np.float32)
    W_out = np.asarray(W_out, np.float32)
    scale = D ** -0.5
    W6 = W_qkv.reshape(DIM, 6, H, D)
    in_maps = []
    xT = [np.ascontiguousarray(x[b].T).astype(np.float16) for b in range(2)]
    for c in range(NCORES):
        b, h0 = c // 4, 4 * (c % 4)
        Wc = W6[:, :, h0:h0 + HPC, :].transpose(0, 2, 1, 3).copy()  # [DIM,4,6,D]
        Wc[:, :, 0, :] *= scale  # qu
        Wc[:, :, 3, :] *= scale  # qc
        Wc[:, :, 2, :] *= 0.5    # vu -> Su' = Su/2 for silu-via-tanh
        # device c-order per head: [qu, vu, ku, qc, vc, kc]
        Wc = Wc[:, :, [0, 2, 1, 3, 5, 4], :]
        wo_c = np.ascontiguousarray(
            W_out.reshape(H, D, DIM)[h0:h0 + HPC].reshape(HPC * D, DIM))
        in_maps.append({
            "xT": xT[b],
            "wq": np.ascontiguousarray(
                Wc.reshape(DIM, HPC * WHEAD)).astype(np.float16),
            "wo": wo_c.astype(np.float16),
        })
    return in_maps


def unshard_output(results):
    """results: list of 8 dicts with 'out0'/'out1' [1024, 2048] partials."""
    outs = []
    for b in range(2):
        acc = np.zeros((DIM, NTOK), np.float32)
        for c in range(4 * b, 4 * b + 4):
            acc += results[c]["out0"].astype(np.float32)
            acc += results[c]["out1"].astype(np.float32)
        outs.append(acc.T)
    return np.stack(outs).astype(np.float32)


def kernel(x, W_qkv, W_out):
    from concourse.bass_utils import run_bass_kernel_spmd

    in_maps = shard_inputs(x, W_qkv, W_out)
    nc = build_nc()
    res = run_bass_kernel_spmd(nc, in_maps, core_ids=list(range(NCORES)))
    return unshard_output(res.results)


# revision 13
# speedup vs baseline: 1.0261x; 1.0261x over previous
"""Trainium2 Bass kernel for nn_Castle_34351148433552 (sparse_attention).

Sharding: 8 cores = 2 batches x 4 head-groups. Core c handles batch c//4,
heads 4*(c%4) .. 4*(c%4)+3. W_qkv is sliced column-wise per head group (with
the q-scale, and the silu-via-tanh 0.5 factor on vu, folded in on the host),
W_out row-wise. Each core emits TWO partial transposed projections ([1024,
2048] fp16, one per head pair); the host sums the 8 partials per batch and
transposes back.

All matmul operands are fp16 (1 PE row/cycle; fp32 accumulation in PSUM).
x^T and W_qkv stay SBUF-resident for the whole kernel. The Su / Sc / exp /
AV chain only computes the causal (i >= k, i >= j) free-ranges. The head
pair {0,1} output projection is emitted inside head 2 so only the {2,3}
pass trails the attention compute.

Device algorithm per (core, head), all in [d|c, n]-transposed layout:
  qkvT = (Wq_head^T x^T) via PE;  LT[j,k] = sigmoid(ku_j . qu_s_k) (j>k);
  T1T[j,i] = vu_j . qc_s_i (i>=j, pre-halved);  SuT'[k,i] = sum_j LT*T1T;
  silu(Su) = Su'*(1+tanh(Su')) (tanh shares the exp ACT table-set);
  scoresT = ScT - silu, causal fill -1e30;  expT;  AV via [vc|1] lhsT gives
  unnormalized out^T plus the softmax denominator in one PSUM accumulation;
  normalize (reciprocal_approx_fast + PE broadcast); project through W_out.
"""

import os
import sys

import numpy as np

for _p in ("/opt/trn_rl_repo", os.path.expanduser("~/.axon_site/_ro/trn_rl_repo")):
    if os.path.isdir(_p) and _p not in sys.path:
        sys.path.insert(0, _p)
        break

H, D, NTOK, DIM = 16, 64, 2048, 1024
P = 128
NB = NTOK // P  # 16 row blocks
GW = 512        # i/k group width
NG = NTOK // GW  # 4 groups
HPC = 4         # heads per core
NCORES = 8
WHEAD = 6 * D   # 384 qkv columns per head


def _lt_offsets():
    off = {}
    o = 0
    for J in range(NB):
        for kg in range(J // 4 + 1):
            w = GW if kg < J // 4 else (J % 4 + 1) * P
            off[(J, kg)] = (o, w)
            o += w
    return off, o


_NC_CACHE = None


def build_nc():
    global _NC_CACHE
    if _NC_CACHE is not None:
        return _NC_CACHE

    import concourse.mybir as mybir
    import concourse.tile as tile
    from concourse import bacc
    from concourse.masks import make_identity

    dt = mybir.dt
    F32 = dt.float32
    F32R = dt.float32r
    F16 = dt.float16
    AF = mybir.ActivationFunctionType
    ALU = mybir.AluOpType

    nc = bacc.Bacc(None, target_bir_lowering=False, debug=False)
    xT_d = nc.dram_tensor("xT", [DIM, NTOK], F16, kind="ExternalInput")
    wq_d = nc.dram_tensor("wq", [DIM, HPC * WHEAD], F16, kind="ExternalInput")
    wo_d = nc.dram_tensor("wo", [HPC * D, DIM], F16, kind="ExternalInput")
    out_d = [nc.dram_tensor(f"out{i}", [DIM, NTOK], F16, kind="ExternalOutput")
             for i in range(2)]

    lt_off, LTW = _lt_offsets()

    with tile.TileContext(nc) as tc:
        with (
            tc.tile_pool(name="const", bufs=1) as constp,
            tc.tile_pool(name="res", bufs=1) as resp,
            tc.tile_pool(name="work", bufs=2) as workp,
            tc.tile_pool(name="outsb", bufs=3) as outsbp,
            tc.tile_pool(name="ps", bufs=6, space="PSUM") as psp,
            tc.tile_pool(name="pavp", bufs=2, space="PSUM") as pavp,
        ):
            # ---------- constants ----------
            ident = constp.tile([P, P], F16, tag="ident")
            make_identity(nc, ident)
            # dmask[p, c] = 1.0 iff c >= p  (T1T diag block: keep i >= j)
            dmask = constp.tile([P, P], F16, tag="dmask")
            nc.gpsimd.memset(dmask, 1.0)
            nc.gpsimd.affine_select(
                out=dmask, in_=dmask, compare_op=ALU.is_ge, fill=0.0,
                base=0, pattern=[[1, P]], channel_multiplier=-1,
            )
            # smask[p, c] = 1.0 iff p > c  (LT diag block: keep j > k strictly)
            smask = constp.tile([P, P], F16, tag="smask")
            nc.gpsimd.memset(smask, 1.0)
            nc.gpsimd.affine_select(
                out=smask, in_=smask, compare_op=ALU.is_gt, fill=0.0,
                base=0, pattern=[[-1, P]], channel_multiplier=1,
            )
            # [vc | 1] stationary blocks, one [128, 65] slot per k-block.
            vc_ones = constp.tile([P, NB * (D + 1)], F16, tag="vco")
            ones_stage = constp.tile([P, NB], F16, tag="onesstage")
            nc.gpsimd.memset(ones_stage, 1.0)
            nc.vector.tensor_copy(
                vc_ones.rearrange("p (k c) -> p k c", c=D + 1)[:, :, D],
                ones_stage)

            # ---------- resident tiles ----------
            xres = resp.tile([P, 8 * NTOK], F16, tag="xres")
            wqres = resp.tile([P, 8 * HPC * WHEAD], F16, tag="wqres")
            # per-head transposed qkv (matmul lhsT/rhs base partitions must
            # match): t0=[qu|vu], t1=[ku|qc], t2=[vc|kc]
            qkvT = [resp.tile([P, NTOK], F16, tag=f"qkvT{i}", name=f"qkvT{i}")
                    for i in range(3)]
            LT = resp.tile([P, LTW], F16, tag="LT")
            T1T = resp.tile([P, NB * GW], F16, tag="T1T")
            attn = [resp.tile([P, NTOK], F16, tag=f"attn{i}", name=f"attn{i}")
                    for i in range(2)]
            wores = [resp.tile([P, DIM], F16, tag=f"wores{i}", name=f"wores{i}")
                     for i in range(2)]

            for i in range(2):
                nc.sync.dma_start(wores[i], wo_d[i * P:(i + 1) * P, :])
            # head-0 weights + first x chunk first so phase A starts early
            wqres3 = wqres.rearrange("p (kc w) -> p kc w", kc=8)
            wqd3 = wq_d.rearrange("(kc p) w -> p kc w", p=P)
            xres3 = xres.rearrange("p (kc n) -> p kc n", kc=8)
            xTd3 = xT_d.rearrange("(kc p) n -> p kc n", p=P)
            nc.sync.dma_start(wqres3[:, :, 0:WHEAD], wqd3[:, :, 0:WHEAD])
            nc.sync.dma_start(
                xres3[:, :, 0:GW], xTd3[:, :, 0:GW])
            for ng in range(1, NG):
                nc.sync.dma_start(
                    xres3[:, :, ng * GW:(ng + 1) * GW],
                    xTd3[:, :, ng * GW:(ng + 1) * GW])
            for h2 in range(1, HPC):
                nc.sync.dma_start(
                    wqres3[:, :, h2 * WHEAD:(h2 + 1) * WHEAD],
                    wqd3[:, :, h2 * WHEAD:(h2 + 1) * WHEAD])

            def emit_out_proj(pair, ngs=range(NG)):
                """Project attn[pair] through wores[pair] into out_d[pair]."""
                for ng in ngs:
                    for dt_ in range(8):
                        pp = psp.tile([P, GW], F32, tag="ps")
                        nc.tensor.matmul(
                            pp, wores[pair][:, dt_ * P:(dt_ + 1) * P],
                            attn[pair][:, ng * GW:(ng + 1) * GW],
                            start=True, stop=True)
                        ot = outsbp.tile([P, GW], F16, tag="ot")
                        if dt_ % 2 == 0:
                            nc.scalar.copy(ot, pp)
                        else:
                            nc.vector.tensor_copy(ot, pp)
                        nc.sync.dma_start(
                            out_d[pair][dt_ * P:(dt_ + 1) * P,
                                        ng * GW:(ng + 1) * GW], ot)

            for hh in range(HPC):
                # ---------- A: qkv projection for this head ----------
                for ng in range(NG):
                    ps = []
                    for ct in range(3):
                        ps.append(psp.tile([P, GW], F32, tag="ps", name=f"psq{ct}"))
                    for kc in range(8):
                        for ct in range(3):
                            wslice = wqres[
                                :, kc * HPC * WHEAD + hh * WHEAD + ct * P:
                                kc * HPC * WHEAD + hh * WHEAD + (ct + 1) * P]
                            nc.tensor.matmul(
                                ps[ct], wslice,
                                xres[:, kc * NTOK + ng * GW:
                                     kc * NTOK + (ng + 1) * GW],
                                start=(kc == 0), stop=(kc == 7),
                            )
                    for ct in range(3):
                        nc.scalar.copy(
                            qkvT[ct][:, ng * GW:(ng + 1) * GW], ps[ct])

                # out-projection of the finished head pair {0,1}, placed
                # after head-2 phase A so the PE never waits for it
                if hh == 2:
                    emit_out_proj(0)

                # ---------- C: LT = masked sigmoid(ku . qu_s) ----------
                # J-ascending so Su of early i-groups can start (subtile deps)
                for J in range(NB):
                    for kg in range(J // 4 + 1):
                        off, w = lt_off[(J, kg)]
                        pl = psp.tile([P, GW], F32, tag="ps")
                        nc.tensor.matmul(
                            pl[:, :w],
                            qkvT[1][0:D, J * P:(J + 1) * P],
                            qkvT[0][0:D, kg * GW: kg * GW + w],
                            start=True, stop=True,
                        )
                        nc.scalar.activation(
                            LT[:, off:off + w], pl[:, :w], AF.Sigmoid)
                        if kg == J // 4:
                            # strict j > k mask on the final (diagonal) block
                            nc.gpsimd.tensor_tensor(
                                LT[:, off + w - P:off + w],
                                LT[:, off + w - P:off + w],
                                smask, op=ALU.mult)

                # ---------- B: vc natural layout + ones columns ----------
                for kb in range(NB):
                    pt = psp.tile([P, D], F16, tag="ps")
                    nc.tensor.transpose(
                        pt, qkvT[2][0:D, kb * P:(kb + 1) * P], ident[0:D, 0:D])
                    nc.vector.tensor_copy(
                        vc_ones[:, kb * (D + 1): kb * (D + 1) + D], pt)

                # ---------- D: attention, per 512-wide i-group ----------
                avsH = workp.tile([D, NTOK], F16, tag="avsH")
                for ig in range(NG):
                    nblk = 4 * ig + 4

                    # T1T strips (term1 transposed, pre-halved); strip J only
                    # needs i >= j, so diag-group strips start at their block
                    for J in range(nblk):
                        o = max(J - 4 * ig, 0) * P  # first needed i-col
                        pt2 = psp.tile([P, GW], F32, tag="ps")
                        nc.tensor.matmul(
                            pt2[:, o:],
                            qkvT[0][D:2 * D, J * P:(J + 1) * P],
                            qkvT[1][D:2 * D, ig * GW + o:(ig + 1) * GW],
                            start=True, stop=True,
                        )
                        dst = T1T[:, J * GW:(J + 1) * GW]
                        if J < 4 * ig:
                            nc.vector.tensor_copy(dst, pt2)
                        else:
                            # diagonal 128-block: keep i >= j
                            nc.vector.tensor_tensor(
                                dst[:, o:o + P], pt2[:, o:o + P], dmask,
                                op=ALU.mult)
                            if o + P < GW:
                                nc.scalar.copy(dst[:, o + P:], pt2[:, o + P:])

                    # head-3 ig>0: project the previous ig's finished
                    # attn[1] columns while this ig computes (placed after the
                    # T1T matmuls so the PE queue never waits on the at-mult)
                    if hh == 3 and ig > 0:
                        emit_out_proj(1, ngs=[ig - 1])

                    # merged scores pass per k-block: Su' accumulate, Sc,
                    # silu-via-tanh, subtract, causal fill, exp, AV accum.
                    # Diagonal k-blocks only need i-cols >= k.
                    pav_t = pavp.tile([D + 1, GW], F32, tag="av")
                    LAG = 2
                    av_q = []

                    def emit_av(K, ext, ko):
                        nc.tensor.matmul(
                            pav_t[:, ko:],
                            vc_ones[:, K * (D + 1):(K + 1) * (D + 1)],
                            ext[:, ko:],
                            start=(K == 0), stop=(K == nblk - 1),
                        )

                    for K in range(nblk):
                        ko = max(K - 4 * ig, 0) * P  # first causal i-col
                        psu = psp.tile([P, GW], F32, tag="ps")
                        for J in range(K, nblk):
                            jo = max(J - 4 * ig, 0) * P
                            o_, _w = lt_off[(J, K // 4)]
                            nc.tensor.matmul(
                                psu[:, jo:],
                                LT[:, o_ + (K % 4) * P: o_ + (K % 4 + 1) * P],
                                T1T[:, J * GW + jo:(J + 1) * GW],
                                start=(J == K), stop=(J == nblk - 1),
                            )
                        psc = psp.tile([P, GW], F32, tag="ps")
                        nc.tensor.matmul(
                            psc[:, ko:],
                            qkvT[2][D:2 * D, K * P:(K + 1) * P],
                            qkvT[1][D:2 * D, ig * GW + ko:(ig + 1) * GW],
                            start=True, stop=True,
                        )
                        tnh = workp.tile([P, GW], F32, tag="tanh")
                        nc.scalar.activation(tnh[:, ko:], psu[:, ko:], AF.Tanh)
                        # silu(Su): tnh = (tnh + 1) * Su'
                        nc.vector.scalar_tensor_tensor(
                            out=tnh[:, ko:], in0=tnh[:, ko:], scalar=1.0,
                            in1=psu[:, ko:], op0=ALU.add, op1=ALU.mult,
                        )
                        sct = workp.tile([P, GW], F32, tag="sct")
                        nc.vector.tensor_tensor(
                            sct[:, ko:], psc[:, ko:], tnh[:, ko:],
                            op=ALU.subtract)
                        if K >= 4 * ig:
                            # partial diag 128-block: keep i >= k
                            nc.gpsimd.affine_select(
                                out=sct[:, ko:ko + P], in_=sct[:, ko:ko + P],
                                compare_op=ALU.is_ge, fill=-1e30,
                                base=0, pattern=[[1, P]], channel_multiplier=-1,
                            )
                        ext = workp.tile([P, GW], F16, tag="ext", bufs=4)
                        nc.scalar.activation(ext[:, ko:], sct[:, ko:], AF.Exp)
                        av_q.append((K, ext, ko))
                        if len(av_q) > LAG:
                            emit_av(*av_q.pop(0))
                    for item in av_q:
                        emit_av(*item)

                    # normalize: stage denom to SBUF (the custom-DVE
                    # reciprocal must not read PSUM), fast reciprocal, fp16,
                    # GpSimd partition-broadcast, all-SBUF fp16 multiply.
                    # The PE queue is never involved.
                    nc.scalar.copy(avsH[:, ig * GW:(ig + 1) * GW], pav_t[0:D, :])
                    dstrow = workp.tile([1, GW], F32, tag="dst", bufs=3)
                    nc.vector.tensor_copy(dstrow, pav_t[D:D + 1, :])
                    recip_t = workp.tile([1, GW], F32, tag="recip", bufs=3)
                    nc.vector.reciprocal_approx_fast(out=recip_t, in_=dstrow)
                    recip16 = workp.tile([1, GW], F16, tag="recip16", bufs=3)
                    nc.vector.tensor_copy(recip16, recip_t)
                    rbc = workp.tile([D, GW], F16, tag="rbc", bufs=3)
                    nc.gpsimd.partition_broadcast(rbc, recip16)
                    at = attn[hh // 2][(hh % 2) * D:(hh % 2 + 1) * D,
                                       ig * GW:(ig + 1) * GW]
                    nc.vector.tensor_tensor(
                        at, avsH[:, ig * GW:(ig + 1) * GW], rbc, op=ALU.mult)

            # ---------- E: trailing columns of head pair {2,3} ----------
            emit_out_proj(1, ngs=[NG - 1])

    nc.compile()
    _NC_CACHE = nc
    return nc


def shard_inputs(x, W_qkv, W_out):
    """Host-side sharding: per-core input dicts (fp16)."""
    x = np.asarray(x, np.float32)
    W_qkv = np.asarray(W_qkv, np.float32)
    W_out = np.asarray(W_out, np.float32)
    scale = D ** -0.5
    W6 = W_qkv.reshape(DIM, 6, H, D)
    in_maps = []
    xT = [np.ascontiguousarray(x[b].T).astype(np.float16) for b in range(2)]
    for c in range(NCORES):
        b, h0 = c // 4, 4 * (c % 4)
        Wc = W6[:, :, h0:h0 + HPC, :].transpose(0, 2, 1, 3).copy()  # [DIM,4,6,D]
        Wc[:, :, 0, :] *= scale  # qu
        Wc[:, :, 3, :] *= scale  # qc
        Wc[:, :, 2, :] *= 0.5    # vu -> Su' = Su/2 for silu-via-tanh
        # device c-order per head: [qu, vu, ku, qc, vc, kc]
        Wc = Wc[:, :, [0, 2, 1, 3, 5, 4], :]
        wo_c = np.ascontiguousarray(
            W_out.reshape(H, D, DIM)[h0:h0 + HPC].reshape(HPC * D, DIM))
        in_maps.append({
            "xT": xT[b],
            "wq": np.ascontiguousarray(
                Wc.reshape(DIM, HPC * WHEAD)).astype(np.float16),
            "wo": wo_c.astype(np.float16),
        })
    return in_maps


def unshard_output(results):
    """results: list of 8 dicts with 'out0'/'out1' [1024, 2048] partials."""
    outs = []
    for b in range(2):
        acc = np.zeros((DIM, NTOK), np.float32)
        for c in range(4 * b, 4 * b + 4):
            acc += results[c]["out0"].astype(np.float32)
            acc += results[c]["out1"].astype(np.float32)
        outs.append(acc.T)
    return np.stack(outs).astype(np.float32)


def kernel(x, W_qkv, W_out):
    from concourse.bass_utils import run_bass_kernel_spmd

    in_maps = shard_inputs(x, W_qkv, W_out)
    nc = build_nc()
    res = run_bass_kernel_spmd(nc, in_maps, core_ids=list(range(NCORES)))
    return unshard_output(res.results)
